# revision 1
# baseline (speedup 1.0000x reference)
"""SSN calc-assoc Trainium2 kernel (nn_CalcAssoc_53145925321404).

Strategy
--------
Host: for each batch image, bucket-sort the 65536 pixels by their center
superpixel id m (index_map value), pad each bucket to a multiple of 16 so
every 16-pixel partition-group is bucket-uniform, and split slots across
2 cores per batch (8 cores total, B=4).

Device (per core, all math on device except integer index/layout prep and
the tiny 256-entry snorm table):
  - One fused GEMM per 128-pixel tile (288 tiles):
      lhsT = [pixel_feats; pixel_feats^2]  (squares computed on-device, DVE)
      rhs  = W = [spixel_feats; -0.5]      ([128, 256], fp32r)
      out[p, n] = dot(pix_p, s_n) - 0.5*||pix_p||^2
  - gpsimd ap_gather pulls each pixel's 9 neighbor columns (bucket-uniform
    per 16-partition group, so the group-shared index lists are exact).
  - Epilogue: dist = -2*out_gathered + table, where table = snorm[nidx] for
    valid neighbors and 1e16 for invalid ones (1e16 - small == 1e16 in f32,
    bit-exact with the reference's INVALID_DIST).
Host: inverse-permute the per-slot results back to [B, 9, H, W].
"""
import numpy as np
from contextlib import ExitStack

import concourse.bacc as bacc
import concourse.tile as tile
from concourse import mybir
from concourse.bass_utils import run_bass_kernel_spmd

# problem constants (hardcoded per harness contract)
B, C, H, W = 4, 64, 256, 256
HW = H * W
NW, NH = 16, 16
NSP = NW * NH            # 256 superpixels
INVALID = np.float32(1e16)

# kernel layout constants
NT = 288                 # 128-pixel tiles per core
CAP = NT * 128           # 36864 slot capacity per core
CHUNK = 32               # tiles per pipeline chunk
NCH = NT // CHUNK        # 9 chunks
GIDX = 6                 # gather indices per slot (3 rows x 2 aligned pairs, d=2)
OWID = 12                # gathered values per slot (6 runs x 2)
IW = CHUNK * GIDX        # indices per chunk (192)
GW = CHUNK * OWID        # gather output width per chunk (384)

# 3x3 neighbor tables: k -> (dy, dx)
_OFFS = np.arange(9)
_DY = _OFFS // 3 - 1
_DX = _OFFS % 3 - 1

_nc_cache = {}


def _neighbor_tables():
    """col[k, j]: clipped neighbor spixel id; valid[k, j]: in-grid mask."""
    j = np.arange(NSP)
    gx = j % NW
    gy = j // NW
    nx = gx[None, :] + _DX[:, None]          # [9, 256]
    ny = gy[None, :] + _DY[:, None]
    valid = (nx >= 0) & (nx < NW) & (ny >= 0) & (ny < NH)
    col = np.clip(ny, 0, NH - 1) * NW + np.clip(nx, 0, NW - 1)
    return col.astype(np.int64), valid


def _build_bass(loop_r=None):
    """Compile the per-core program (shared by all 8 cores)."""
    key = loop_r
    if key in _nc_cache:
        return _nc_cache[key]
    nc = bacc.Bacc("TRN2", target_bir_lowering=False, debug=False, num_devices=8)
    x_in = nc.dram_tensor("x", [C, CAP], mybir.dt.float32, kind="ExternalInput").ap()
    w_in = nc.dram_tensor("w", [128, NSP], mybir.dt.float32, kind="ExternalInput").ap()
    i_in = nc.dram_tensor("i", [128, NCH, IW // 16], mybir.dt.int16, kind="ExternalInput").ap()
    t_in = nc.dram_tensor("t", [128, NCH, GW], mybir.dt.float32, kind="ExternalInput").ap()
    o_out = nc.dram_tensor("o", [128, NCH * GW], mybir.dt.float32, kind="ExternalOutput").ap()

    with tile.TileContext(nc) as tc, ExitStack() as ctx:
        const_pool = ctx.enter_context(tc.tile_pool(name="const", bufs=1))
        stat_pool = ctx.enter_context(tc.tile_pool(name="stat", bufs=3))
        src_pool = ctx.enter_context(tc.tile_pool(name="src", bufs=3))
        idx_pool = ctx.enter_context(tc.tile_pool(name="idx", bufs=1))
        tbl_pool = ctx.enter_context(tc.tile_pool(name="tbl", bufs=3))
        g_pool = ctx.enter_context(tc.tile_pool(name="g", bufs=4))
        out_pool = ctx.enter_context(tc.tile_pool(name="out", bufs=4))
        psum_pool = ctx.enter_context(tc.tile_pool(name="psum", bufs=4, space="PSUM"))

        wt = const_pool.tile([128, NSP], mybir.dt.float32r)
        nc.sync.dma_start(wt[:], w_in[:].bitcast(mybir.dt.float32r))
        idxt = idx_pool.tile([128, NCH, IW // 16], mybir.dt.int16)
        nc.sync.dma_start(idxt[:], i_in[:])

        def body(_iv):
            for ch in range(NCH):
                x_sl = x_in[:, ch * CHUNK * 128:(ch + 1) * CHUNK * 128]
                stat = stat_pool.tile([128, CHUNK * 128], mybir.dt.float32r)
                nc.sync.dma_start(stat[0:C, :], x_sl.bitcast(mybir.dt.float32r))
                nc.vector.tensor_mul(stat[C:128, :],
                                     stat[0:C, :].bitcast(mybir.dt.float32),
                                     stat[0:C, :].bitcast(mybir.dt.float32))
                tblt = tbl_pool.tile([128, GW], mybir.dt.float32)
                nc.sync.dma_start(tblt[:], t_in[:, ch, :])
                src = src_pool.tile([128, CHUNK * NSP], mybir.dt.float32)
                for sp in range(CHUNK // 4):
                    pt = psum_pool.tile([128, 4 * NSP], mybir.dt.float32)
                    for q in range(4):
                        s = 4 * sp + q
                        nc.tensor.matmul(pt[:, q * NSP:(q + 1) * NSP],
                                         stat[:, s * 128:(s + 1) * 128], wt[:],
                                         start=True, stop=True)
                    dst = src[:, 4 * sp * NSP:(4 * sp + 4) * NSP]
                    if sp % 4 < 3:
                        nc.vector.tensor_copy(dst, pt[:])
                    else:
                        nc.scalar.copy(dst, pt[:])
                g1 = g_pool.tile([128, GW], mybir.dt.float32)
                nc.gpsimd.ap_gather(g1[:], src[:], idxt[:, ch, :], channels=128,
                                    num_elems=CHUNK * NSP // 2, d=2, num_idxs=IW)
                ot = out_pool.tile([128, GW], mybir.dt.float32)
                nc.vector.tensor_scalar(ot[:], g1[:], -2.0, None,
                                        op0=mybir.AluOpType.mult)
                nc.vector.tensor_add(ot[:], ot[:], tblt[:])
                nc.sync.dma_start(o_out[:, ch * GW:(ch + 1) * GW], ot[:])

        if loop_r is None:
            body(None)
        else:
            with tc.For_i(0, loop_r, 1) as iv:
                body(iv)
    nc.compile()
    _nc_cache[key] = nc
    return nc


def _prep_core_inputs(pixel_feats, spixel_feats, index_map):
    """Sort/bucket/pad on host; build the 8 per-core input maps plus the
    unsort metadata (slot -> original pixel id)."""
    col_tab, valid_tab = _neighbor_tables()
    in_maps = []
    meta = []
    for b in range(B):
        m = np.asarray(index_map[b]).reshape(-1).astype(np.int64)
        order = np.argsort(m, kind="stable")
        counts = np.bincount(m, minlength=NSP)
        pad_counts = ((counts + 15) // 16) * 16
        total = int(pad_counts.sum())
        # slot arrays for the whole batch
        slot_px = np.full(total, -1, dtype=np.int64)
        slot_bucket = np.repeat(np.arange(NSP), pad_counts)
        off_pad = np.concatenate([[0], np.cumsum(pad_counts)[:-1]])
        off_real = np.concatenate([[0], np.cumsum(counts)[:-1]])
        pos = off_pad[m[order]] + (np.arange(HW) - off_real[m[order]])
        slot_px[pos] = order
        # split across the batch's two cores at a 16-aligned point
        split = min(CAP, ((total // 2 + 15) // 16) * 16)
        assert split <= CAP and (total - split) <= CAP
        feats = np.asarray(pixel_feats[b]).reshape(C, HW)
        snorm = (np.asarray(spixel_feats[b]).astype(np.float64) ** 2).sum(0)
        snorm = snorm.astype(np.float32)
        w_full = np.concatenate(
            [np.asarray(spixel_feats[b]).astype(np.float32),
             np.full((C, NSP), -0.5, dtype=np.float32)], axis=0)
        for half, (lo, hi) in enumerate(((0, split), (split, total))):
            n = hi - lo
            spx = np.full(CAP, -1, dtype=np.int64)
            sbk = np.zeros(CAP, dtype=np.int64)
            spx[:n] = slot_px[lo:hi]
            sbk[:n] = slot_bucket[lo:hi]
            xs = np.zeros((C, CAP), dtype=np.float32)
            real = spx >= 0
            xs[:, real] = feats[:, spx[real]]
            # group-uniform bucket per (tile, group)
            gb = sbk.reshape(NT * 8, 16)
            assert (gb == gb[:, :1]).all(), "16-slot group not bucket-uniform"
            gbt = gb[:, 0].reshape(NT, 8)          # [tile, group] -> bucket
            # d=2 run gather: per slot, 3 rows x 2 aligned pairs.
            s_of_t = np.arange(NT) % CHUNK
            cols = col_tab[:, gbt]                  # [9, NT, 8] clipped col ids
            vt = valid_tab[:, gbt]                  # [9, NT, 8]
            # row-center col per (r=dy_index, tile, group): k = 3r+1 (dx=0)
            crow = cols.reshape(3, 3, NT, 8)[:, 1]  # [3, NT, 8]
            blk1 = np.maximum(crow - 1, 0) // 2     # [3, NT, 8]
            blk2 = np.minimum(blk1 + 1, NSP // 2 - 1)
            runs = np.stack([blk1, blk2], axis=1)   # [3, 2, NT, 8] (d=2 units)
            vals = (runs + (s_of_t * (NSP // 2))[None, None, :, None])
            vals = vals.transpose(2, 3, 0, 1).reshape(NT, 8, GIDX)
            idx_arr = np.zeros((128, NCH, IW // 16), dtype=np.int16)
            jpos = np.arange(IW)
            for g in range(8):
                per_chunk = vals.reshape(NCH, CHUNK, 8, GIDX)[:, :, g, :]
                flat = per_chunk.reshape(NCH, IW)   # j = s*GIDX + (r*2 + which)
                idx_arr[16 * g + (jpos % 16), :, jpos // 16] = flat.T.astype(np.int16)[jpos]
            # position of (k) inside the 12-wide slot block: r*4 + offset
            ct = cols.reshape(3, 3, NT, 8)          # [r, dxi, NT, 8]
            off1 = ct - 2 * blk1[:, None]           # offset if in run1 (0..1)
            in1 = (off1 >= 0) & (off1 <= 1)
            off = np.where(in1, off1, 2 + ct - 2 * blk2[:, None])
            pos = (np.arange(3)[:, None, None, None] * 4 + off)  # [r, dxi, NT, 8]
            pos = pos.reshape(9, NT, 8)
            # epilogue table [128, NT, OWID]
            tbl = np.full((128, NT, OWID), INVALID, dtype=np.float32)
            sn = snorm[cols]                        # [9, NT, 8]
            for g in range(8):
                for k in range(9):
                    vmask = vt[k, :, g]
                    p_k = pos[k, :, g]
                    t_idx = np.arange(NT)[vmask]
                    tbl[16 * g:16 * (g + 1), t_idx, p_k[vmask]] = sn[k, :, g][None, vmask]
            tbl = tbl.reshape(128, NCH, GW)
            # per-slot k -> column positions + validity for host unsort
            pos_slots = pos.transpose(1, 2, 0)      # [NT, 8, 9]
            pos_cap = np.repeat(pos_slots.reshape(NT * 8, 9), 16, axis=0)  # [CAP, 9]
            val_slots = vt.transpose(1, 2, 0)       # [NT, 8, 9]
            val_cap = np.repeat(val_slots.reshape(NT * 8, 9), 16, axis=0)
            in_maps.append({"x": xs, "w": w_full, "i": idx_arr, "t": tbl})
            meta.append((b, spx, pos_cap, val_cap))
    return in_maps, meta


def kernel(pixel_feats, spixel_feats, index_map, _loop_r=None, _nc=None):
    in_maps, meta = _prep_core_inputs(pixel_feats, spixel_feats, index_map)
    nc = _nc if _nc is not None else _build_bass(_loop_r)
    res = run_bass_kernel_spmd(nc, in_maps, core_ids=list(range(8)))
    out = np.empty((B, 9, HW), dtype=np.float32)
    for (b, spx, pos_cap, val_cap), r in zip(meta, res.results):
        o = r["o"]                                  # [128, NCH*GW]
        arr = o.reshape(128, NCH, CHUNK, OWID).transpose(1, 2, 0, 3)
        arr = arr.reshape(CAP, OWID)                # slot-major
        vals9 = arr[np.arange(CAP)[:, None], pos_cap]  # [CAP, 9]
        vals9 = np.where(val_cap, vals9, INVALID)
        real = spx >= 0
        out[b][:, spx[real]] = vals9[real].T
    return out.reshape(B, 9, H, W)



# revision 2
# speedup vs baseline: 1.0599x; 1.0599x over previous
"""SSN calc-assoc Trainium2 kernel, strip-GEMM, gather-free (option D).

Host: per batch, bucket-sort pixels by center superpixel id, pad buckets to
x16, give each superpixel GRID ROW a fixed quota of 18 tiles (2304 slots)
per core, split each row's slots across the batch's 2 cores. Tile t serves
row ry=t//18 with static 3-row strip [sb, sb+3), sb=clip(ry-1,0,13).

Device per tile: one bf16 matmul computes the full distance to all 48 strip
candidates: lhsT = [x (64); pnorm_hi; pnorm_lo; 1; 1] (K=68, per-slot),
rhs = W[:, 16sb:16sb+48] where W = [-2*s; 1; 1; snorm_hi; snorm_lo].
out[p, n] = ||x_p||^2 - 2<x_p, s_n> + ||s_n||^2. PSUM -> SBUF copy converts
to bf16; DMA the 48-wide strips out. No gather, no table: the host unsort
picks each pixel's 9 neighbor columns and masks off-grid ones with 1e16.
"""
import numpy as np
from contextlib import ExitStack

import concourse.bacc as bacc
import concourse.tile as tile
from concourse import mybir
from concourse.bass_utils import run_bass_kernel_spmd

import ml_dtypes

B, C, H, W = 4, 64, 256, 256
HW = H * W
NW, NH = 16, 16
NSP = NW * NH
INVALID = np.float32(1e16)

K = 68                   # 64 feats + pnorm hi/lo + const 1,1 (snorm rows)
NT = 288                 # tiles per core
TPR = NT // NH           # tiles per grid row = 18
RCAP = TPR * 128         # slot capacity per (core,row) = 2304
CAP = NT * 128           # 36864
CHUNK = 32               # tiles per chunk
NCH = NT // CHUNK        # 9
SW = 48                  # strip width (3 rows x 16)

_STRIP_BASE = np.clip(np.arange(NT) // TPR - 1, 0, NH - 3)   # per tile

_nc_cache = {}


def _build_bass(loop_r=None):
    key = loop_r
    if key in _nc_cache:
        return _nc_cache[key]
    nc = bacc.Bacc("TRN2", target_bir_lowering=False, debug=False, num_devices=8)
    BF = mybir.dt.bfloat16
    x_in = nc.dram_tensor("x", [K, CAP], BF, kind="ExternalInput").ap()
    w_in = nc.dram_tensor("w", [K, NSP], BF, kind="ExternalInput").ap()
    o_out = nc.dram_tensor("o", [128, NT * SW], BF, kind="ExternalOutput").ap()

    with tile.TileContext(nc) as tc, ExitStack() as ctx:
        const_pool = ctx.enter_context(tc.tile_pool(name="const", bufs=1))
        stat_pool = ctx.enter_context(tc.tile_pool(name="stat", bufs=3))
        out_pool = ctx.enter_context(tc.tile_pool(name="out", bufs=3))
        psum_pool = ctx.enter_context(tc.tile_pool(name="psum", bufs=8, space="PSUM"))

        wt = const_pool.tile([K, NSP], BF)
        nc.sync.dma_start(wt[:], w_in[:])

        def body(_iv):
            # loads issue from the SP (sync) DGE queue; stores from the ACT
            # (scalar) DGE queue so a store's wait-on-compute never blocks
            # the next chunk's input load. All PSUM->SBUF copies on DVE.
            for ch in range(NCH):
                stat = stat_pool.tile([K, CHUNK * 128], BF)
                nc.sync.dma_start(stat[:], x_in[:, ch * CHUNK * 128:(ch + 1) * CHUNK * 128])
                ot = out_pool.tile([128, CHUNK * SW], BF)
                for grp in range(CHUNK // 4):
                    pt = psum_pool.tile([128, 4 * SW], mybir.dt.float32)
                    for q in range(4):
                        tl = grp * 4 + q
                        sb = int(_STRIP_BASE[ch * CHUNK + tl])
                        nc.tensor.matmul(pt[:, q * SW:(q + 1) * SW],
                                         stat[:, tl * 128:(tl + 1) * 128],
                                         wt[:, 16 * sb:16 * sb + SW],
                                         start=True, stop=True)
                    dst = ot[:, grp * 4 * SW:(grp + 1) * 4 * SW]
                    nc.vector.tensor_copy(dst, pt[:])
                nc.scalar.dma_start(
                    o_out[:, ch * CHUNK * SW:(ch + 1) * CHUNK * SW], ot[:])

        if loop_r is None:
            body(None)
        else:
            with tc.For_i(0, loop_r, 1) as iv:
                body(iv)
    nc.compile()
    _nc_cache[key] = nc
    return nc


def _prep_core_inputs(pixel_feats, spixel_feats, index_map):
    in_maps = []
    meta = []
    for b in range(B):
        m = np.asarray(index_map[b]).reshape(-1).astype(np.int64)
        order = np.argsort(m, kind="stable")
        counts = np.bincount(m, minlength=NSP)
        pad_counts = ((counts + 15) // 16) * 16
        off_pad = np.concatenate([[0], np.cumsum(pad_counts)[:-1]])
        off_real = np.concatenate([[0], np.cumsum(counts)[:-1]])
        total = int(pad_counts.sum())
        slot_px = np.full(total, -1, dtype=np.int64)
        slot_bucket = np.repeat(np.arange(NSP), pad_counts)
        pos = off_pad[m[order]] + (np.arange(HW) - off_real[m[order]])
        slot_px[pos] = order

        feats = np.asarray(pixel_feats[b]).reshape(C, HW)
        feats_bf = feats.astype(ml_dtypes.bfloat16)
        s64 = np.asarray(spixel_feats[b]).astype(np.float64)
        snorm = (s64 ** 2).sum(0)
        w_full = np.zeros((K, NSP), dtype=np.float64)
        w_full[:C] = -2.0 * s64
        w_full[C] = 1.0
        w_full[C + 1] = 1.0
        sn_hi = snorm.astype(ml_dtypes.bfloat16)
        sn_lo = (snorm - sn_hi.astype(np.float64)).astype(ml_dtypes.bfloat16)
        w_bf = w_full.astype(ml_dtypes.bfloat16)
        w_bf[C + 2] = sn_hi
        w_bf[C + 3] = sn_lo

        row_pc = pad_counts.reshape(NH, NW).sum(1)
        row_lo = np.concatenate([[0], np.cumsum(row_pc)[:-1]])
        for half in range(2):
            spx = np.full(CAP, -1, dtype=np.int64)
            sbk = np.zeros(CAP, dtype=np.int64)
            for ry in range(NH):
                n_row = int(row_pc[ry])
                cut = min(RCAP, ((n_row // 2 + 15) // 16) * 16)
                lo, hi = (0, cut) if half == 0 else (cut, n_row)
                seg = slice(int(row_lo[ry]) + lo, int(row_lo[ry]) + hi)
                n = hi - lo
                assert n <= RCAP, (ry, n)
                dst = slice(ry * RCAP, ry * RCAP + n)
                spx[dst] = slot_px[seg]
                sbk[dst] = slot_bucket[seg]
                sbk[ry * RCAP + n: (ry + 1) * RCAP] = ry * NW
            xs = np.zeros((K, CAP), dtype=ml_dtypes.bfloat16)
            real = spx >= 0
            xs[:C, real] = feats_bf[:, spx[real]]
            pn = (xs[:C].astype(np.float64) ** 2).sum(0)
            pn_hi = pn.astype(ml_dtypes.bfloat16)
            pn_lo = (pn - pn_hi.astype(np.float64)).astype(ml_dtypes.bfloat16)
            xs[C] = pn_hi
            xs[C + 1] = pn_lo
            xs[C + 2] = 1.0
            xs[C + 3] = 1.0

            gb = sbk.reshape(NT * 8, 16)
            assert (gb == gb[:, :1]).all(), "16-slot group not bucket-uniform"
            gbt = gb[:, 0].reshape(NT, 8)
            t_row = np.arange(NT) // TPR
            assert (gbt // NW == t_row[:, None]).all(), "tile not row-pure"
            sb_t = _STRIP_BASE
            cy = gbt // NW
            cx = gbt % NW

            # neighbor positions inside the 48-wide strip block
            koffs = np.arange(9)
            dy = koffs // 3 - 1
            dx = koffs % 3 - 1
            ny = cy[None] + dy[:, None, None]            # [9, NT, 8]
            nx = cx[None] + dx[:, None, None]
            valid = (nx >= 0) & (nx < NW) & (ny >= 0) & (ny < NH)
            nyc = np.clip(ny, 0, NH - 1)
            nxc = np.clip(nx, 0, NW - 1)
            i_sec = nyc - sb_t[None, :, None]
            assert ((i_sec >= 0) & (i_sec < 3)).all()
            pos = i_sec * NW + nxc                       # [9, NT, 8] in [0,48)

            pos_slots = pos.transpose(1, 2, 0)
            pos_cap = np.repeat(pos_slots.reshape(NT * 8, 9), 16, axis=0)
            val_slots = valid.transpose(1, 2, 0)
            val_cap = np.repeat(val_slots.reshape(NT * 8, 9), 16, axis=0)
            in_maps.append({"x": xs, "w": w_bf})
            meta.append((b, spx, pos_cap, val_cap))
    return in_maps, meta


def kernel(pixel_feats, spixel_feats, index_map, _loop_r=None, _nc=None):
    in_maps, meta = _prep_core_inputs(pixel_feats, spixel_feats, index_map)
    nc = _nc if _nc is not None else _build_bass(_loop_r)
    res = run_bass_kernel_spmd(nc, in_maps, core_ids=list(range(8)))
    out = np.empty((B, 9, HW), dtype=np.float32)
    for (b, spx, pos_cap, val_cap), r in zip(meta, res.results):
        o = np.asarray(r["o"]).astype(np.float32)        # [128, NT*SW]
        arr = o.reshape(128, NCH, CHUNK, SW).transpose(1, 2, 0, 3)
        arr = arr.reshape(CAP, SW)
        vals9 = arr[np.arange(CAP)[:, None], pos_cap]
        vals9 = np.where(val_cap, vals9, INVALID)
        real = spx >= 0
        out[b][:, spx[real]] = vals9[real].T
    return out.reshape(B, 9, H, W)


# revision 3
# speedup vs baseline: 1.0833x; 1.0220x over previous
"""SSN calc-assoc Trainium2 kernel, strip-GEMM, gather-free (option D).

Host: per batch, bucket-sort pixels by center superpixel id, pad buckets to
x16, give each superpixel GRID ROW a fixed quota of 18 tiles (2304 slots)
per core, split each row's slots across the batch's 2 cores. Tile t serves
row ry=t//18 with static 3-row strip [sb, sb+3), sb=clip(ry-1,0,13).

Device per tile: one bf16 matmul computes the full distance to all 48 strip
candidates: lhsT = [x (64); pnorm_hi; pnorm_lo; 1; 1] (K=68, per-slot),
rhs = W[:, 16sb:16sb+48] where W = [-2*s; 1; 1; snorm_hi; snorm_lo].
out[p, n] = ||x_p||^2 - 2<x_p, s_n> + ||s_n||^2. PSUM -> SBUF copy converts
to bf16; DMA the 48-wide strips out. No gather, no table: the host unsort
picks each pixel's 9 neighbor columns and masks off-grid ones with 1e16.
"""
import numpy as np
from contextlib import ExitStack

import concourse.bacc as bacc
import concourse.tile as tile
from concourse import mybir
from concourse.bass_utils import run_bass_kernel_spmd

import ml_dtypes

B, C, H, W = 4, 64, 256, 256
HW = H * W
NW, NH = 16, 16
NSP = NW * NH
INVALID = np.float32(1e16)

K = 68                   # 64 feats + pnorm hi/lo + const 1,1 (snorm rows)
NT = 288                 # tiles per core
TPR = NT // NH           # tiles per grid row = 18
RCAP = TPR * 128         # slot capacity per (core,row) = 2304
CAP = NT * 128           # 36864
CHUNK = 32               # tiles per chunk
NCH = NT // CHUNK        # 9
SW = 48                  # strip width (3 rows x 16)

_STRIP_BASE = np.clip(np.arange(NT) // TPR - 1, 0, NH - 3)   # per tile

_nc_cache = {}


def _build_bass(loop_r=None):
    key = loop_r
    if key in _nc_cache:
        return _nc_cache[key]
    nc = bacc.Bacc("TRN2", target_bir_lowering=False, debug=False, num_devices=8)
    BF = mybir.dt.bfloat16
    x_in = nc.dram_tensor("x", [K, CAP], BF, kind="ExternalInput").ap()
    w_in = nc.dram_tensor("w", [K, NSP], BF, kind="ExternalInput").ap()
    o_out = nc.dram_tensor("o", [128, NT * SW], BF, kind="ExternalOutput").ap()

    with tile.TileContext(nc) as tc, ExitStack() as ctx:
        const_pool = ctx.enter_context(tc.tile_pool(name="const", bufs=1))
        stat_pool = ctx.enter_context(tc.tile_pool(name="stat", bufs=9))
        out_pool = ctx.enter_context(tc.tile_pool(name="out", bufs=4))
        psum_pool = ctx.enter_context(tc.tile_pool(name="psum", bufs=8, space="PSUM"))

        wt = const_pool.tile([K, NSP], BF)
        nc.sync.dma_start(wt[:], w_in[:])

        def body(_iv):
            # loads issue from the SP (sync) DGE queue; stores from the ACT
            # (scalar) DGE queue so a store's wait-on-compute never blocks
            # the next chunk's input load. All PSUM->SBUF copies on DVE.
            for ch in range(NCH):
                stat = stat_pool.tile([K, CHUNK * 128], BF)
                half = CHUNK * 64
                base = ch * CHUNK * 128
                nc.sync.dma_start(stat[:, :half], x_in[:, base:base + half])
                nc.sync.dma_start(stat[:, half:], x_in[:, base + half:base + CHUNK * 128])
                ot = out_pool.tile([128, CHUNK * SW], BF)
                for grp in range(CHUNK // 4):
                    pt = psum_pool.tile([128, 4 * SW], mybir.dt.float32)
                    for q in range(4):
                        tl = grp * 4 + q
                        sb = int(_STRIP_BASE[ch * CHUNK + tl])
                        nc.tensor.matmul(pt[:, q * SW:(q + 1) * SW],
                                         stat[:, tl * 128:(tl + 1) * 128],
                                         wt[:, 16 * sb:16 * sb + SW],
                                         start=True, stop=True)
                    dst = ot[:, grp * 4 * SW:(grp + 1) * 4 * SW]
                    nc.vector.tensor_copy(dst, pt[:])
                nc.scalar.dma_start(
                    o_out[:, ch * CHUNK * SW:(ch + 1) * CHUNK * SW], ot[:])

        if loop_r is None:
            body(None)
        else:
            with tc.For_i(0, loop_r, 1) as iv:
                body(iv)
    nc.compile()
    _nc_cache[key] = nc
    return nc


def _prep_core_inputs(pixel_feats, spixel_feats, index_map):
    in_maps = []
    meta = []
    for b in range(B):
        m = np.asarray(index_map[b]).reshape(-1).astype(np.int64)
        order = np.argsort(m, kind="stable")
        counts = np.bincount(m, minlength=NSP)
        pad_counts = ((counts + 15) // 16) * 16
        off_pad = np.concatenate([[0], np.cumsum(pad_counts)[:-1]])
        off_real = np.concatenate([[0], np.cumsum(counts)[:-1]])
        total = int(pad_counts.sum())
        slot_px = np.full(total, -1, dtype=np.int64)
        slot_bucket = np.repeat(np.arange(NSP), pad_counts)
        pos = off_pad[m[order]] + (np.arange(HW) - off_real[m[order]])
        slot_px[pos] = order

        feats = np.asarray(pixel_feats[b]).reshape(C, HW)
        feats_bf = feats.astype(ml_dtypes.bfloat16)
        s64 = np.asarray(spixel_feats[b]).astype(np.float64)
        snorm = (s64 ** 2).sum(0)
        w_full = np.zeros((K, NSP), dtype=np.float64)
        w_full[:C] = -2.0 * s64
        w_full[C] = 1.0
        w_full[C + 1] = 1.0
        sn_hi = snorm.astype(ml_dtypes.bfloat16)
        sn_lo = (snorm - sn_hi.astype(np.float64)).astype(ml_dtypes.bfloat16)
        w_bf = w_full.astype(ml_dtypes.bfloat16)
        w_bf[C + 2] = sn_hi
        w_bf[C + 3] = sn_lo

        row_pc = pad_counts.reshape(NH, NW).sum(1)
        row_lo = np.concatenate([[0], np.cumsum(row_pc)[:-1]])
        for half in range(2):
            spx = np.full(CAP, -1, dtype=np.int64)
            sbk = np.zeros(CAP, dtype=np.int64)
            for ry in range(NH):
                n_row = int(row_pc[ry])
                cut = min(RCAP, ((n_row // 2 + 15) // 16) * 16)
                lo, hi = (0, cut) if half == 0 else (cut, n_row)
                seg = slice(int(row_lo[ry]) + lo, int(row_lo[ry]) + hi)
                n = hi - lo
                assert n <= RCAP, (ry, n)
                dst = slice(ry * RCAP, ry * RCAP + n)
                spx[dst] = slot_px[seg]
                sbk[dst] = slot_bucket[seg]
                sbk[ry * RCAP + n: (ry + 1) * RCAP] = ry * NW
            xs = np.zeros((K, CAP), dtype=ml_dtypes.bfloat16)
            real = spx >= 0
            xs[:C, real] = feats_bf[:, spx[real]]
            pn = (xs[:C].astype(np.float64) ** 2).sum(0)
            pn_hi = pn.astype(ml_dtypes.bfloat16)
            pn_lo = (pn - pn_hi.astype(np.float64)).astype(ml_dtypes.bfloat16)
            xs[C] = pn_hi
            xs[C + 1] = pn_lo
            xs[C + 2] = 1.0
            xs[C + 3] = 1.0

            gb = sbk.reshape(NT * 8, 16)
            assert (gb == gb[:, :1]).all(), "16-slot group not bucket-uniform"
            gbt = gb[:, 0].reshape(NT, 8)
            t_row = np.arange(NT) // TPR
            assert (gbt // NW == t_row[:, None]).all(), "tile not row-pure"
            sb_t = _STRIP_BASE
            cy = gbt // NW
            cx = gbt % NW

            # neighbor positions inside the 48-wide strip block
            koffs = np.arange(9)
            dy = koffs // 3 - 1
            dx = koffs % 3 - 1
            ny = cy[None] + dy[:, None, None]            # [9, NT, 8]
            nx = cx[None] + dx[:, None, None]
            valid = (nx >= 0) & (nx < NW) & (ny >= 0) & (ny < NH)
            nyc = np.clip(ny, 0, NH - 1)
            nxc = np.clip(nx, 0, NW - 1)
            i_sec = nyc - sb_t[None, :, None]
            assert ((i_sec >= 0) & (i_sec < 3)).all()
            pos = i_sec * NW + nxc                       # [9, NT, 8] in [0,48)

            pos_slots = pos.transpose(1, 2, 0)
            pos_cap = np.repeat(pos_slots.reshape(NT * 8, 9), 16, axis=0)
            val_slots = valid.transpose(1, 2, 0)
            val_cap = np.repeat(val_slots.reshape(NT * 8, 9), 16, axis=0)
            in_maps.append({"x": xs, "w": w_bf})
            meta.append((b, spx, pos_cap, val_cap))
    return in_maps, meta


def kernel(pixel_feats, spixel_feats, index_map, _loop_r=None, _nc=None):
    in_maps, meta = _prep_core_inputs(pixel_feats, spixel_feats, index_map)
    nc = _nc if _nc is not None else _build_bass(_loop_r)
    res = run_bass_kernel_spmd(nc, in_maps, core_ids=list(range(8)))
    out = np.empty((B, 9, HW), dtype=np.float32)
    for (b, spx, pos_cap, val_cap), r in zip(meta, res.results):
        o = np.asarray(r["o"]).astype(np.float32)        # [128, NT*SW]
        arr = o.reshape(128, NCH, CHUNK, SW).transpose(1, 2, 0, 3)
        arr = arr.reshape(CAP, SW)
        vals9 = arr[np.arange(CAP)[:, None], pos_cap]
        vals9 = np.where(val_cap, vals9, INVALID)
        real = spx >= 0
        out[b][:, spx[real]] = vals9[real].T
    return out.reshape(B, 9, H, W)
